# revision 15
# baseline (speedup 1.0000x reference)
"""CGC (Customized Gate Control) MoE kernel for Trainium2, 8 NeuronCores.

Problem: 3 inputs x_{shared,task1,task2} [4096, 1024]; three expert groups
(sh/t1/t2) of 4 experts each; expert = fc2(relu(fc1(x))) with
fc1: 1024->2048, fc2: 2048->512; three softmax gates; outputs
(out_sh, out1, out2) each [4096, 512] as gate-weighted sums of expert
outputs.

Sharding: data-parallel over batch across 8 cores (512 rows/core), all
weights replicated. No collectives.

v3 design (vs the 522us fp32r v1):
  - weights/x/wg cast to bf16 on the host: halves HBM weight streaming
    (151MB -> 76MB per core-iteration); matmul throughput unchanged (PE
    streams 1 col/cycle for bf16 and fp32r alike), accuracy ~4e-3 rel
    (gate is 2e-2).
  - xT loaded directly with DMA-transpose (16-bit xbar path) on the ACT
    hardware DMA queue: removes 96 PE transposes + 96 PSUM drains and
    unserializes input loads from the weight stream (SP queue).
  - gate softmax chains and per-head bias matmuls are interleaved into
    the first expert's fc1 stream (2 per ht-block) so PE never idles
    waiting for DVE/ACT softmax round-trips.
  - fc2 bias via bias_head = g_head @ B2_head (transposed-gate stationary,
    12 N=512 matmuls) initializing the accumulators, replacing 48
    ones@b2 matmuls.
  - output stores on the gpsimd (software DGE) queue so next-iteration
    input DMAs never queue behind them.

Per-core dataflow (batch tile b=512, partition tiles of 128):
  - xT [1024, 512] bf16 via DMA-transpose (ACT queue)
  - gates: logits = xT.T @ wg + bg (PE) -> softmax (DVE+ACT), batch-major
  - gate transpose gT (PE), acc[h] init = gT.T @ B2 (PE)
  - per expert e: hT[ht] = relu(W1[:,ht].T @ xT + b1) (PE + DVE/ACT), bf16
                  o[bt] += hT[:,bt].T @ W2[ht] over ht (PE, PSUM accum)
                  acc[head][bt] += g[head][:,e] * o[bt] (DVE)
  - store acc -> outputs (gpsimd queue).
"""
import sys
from contextlib import nullcontext

if "/opt/trn_rl_repo" not in sys.path:
    sys.path.insert(0, "/opt/trn_rl_repo")

import numpy as np
import ml_dtypes

import concourse.bass as bass
import concourse.mybir as mybir
from concourse import bacc
from concourse.tile import TileContext
from concourse.masks import make_identity

B, I, H, O = 4096, 1024, 2048, 512
E = 4                      # experts per group
N_CORES = 8
BL = B // N_CORES          # 512 rows per core
BT = BL // 128             # 4 batch tiles
IT = I // 128              # 8 input tiles
HT = H // 128              # 16 hidden tiles

F32 = mybir.dt.float32
F32R = mybir.dt.float32r
BF16 = mybir.dt.bfloat16

GROUPS = ("t1", "sh", "t2")  # sh mid so the last expert tail is 2-contrib
GATE_W = {"sh": 2 * E + E, "t1": E + E, "t2": E + E}  # 12, 8, 8
# head -> (gate group, [(expert group, base col), ...])
HEADS = {
    "o1": ("t1", [("t1", 0), ("sh", E)]),
    "o2": ("t2", [("t2", 0), ("sh", E)]),
    "osh": ("sh", [("t1", 0), ("t2", E), ("sh", 2 * E)]),
}


# (group, e) -> list of (head, gate_name, gate_col)
def _contribs(grp, e):
    # sh-gate columns are host-permuted to [t1, sh, t2]
    if grp == "t1":
        return [("o1", "t1", e), ("osh", "sh", e)]
    if grp == "t2":
        return [("o2", "t2", e), ("osh", "sh", 2 * E + e)]
    return [("o1", "t1", E + e), ("o2", "t2", E + e), ("osh", "sh", E + e)]


def build_nc(loop_reps=None, mode="full", unroll=1, skew=3,
             w1bufs=3, w2bufs=3, htbufs=6, direct_acc=False, gp_tag=False,
             phbufs=4):
    """Build the per-core kernel. loop_reps wraps the whole body in a
    hardware For_i loop; mode selects "full", "dma" (loads only) or
    "compute" (no weight loads) - both diagnostics-only. unroll emits the
    body N times inline (sim-only, for steady-state slope estimates)."""
    nc = bacc.Bacc(None)

    # ---- DRAM parameters ----------------------------------------------
    xs = {g: nc.declare_dram_parameter(f"x_{g}", [BL, I], BF16, isOutput=False)
          for g in GROUPS}
    w1 = {g: nc.declare_dram_parameter(f"w1_{g}", [E, I, H], BF16, isOutput=False)
          for g in GROUPS}
    b1 = {g: nc.declare_dram_parameter(f"b1_{g}", [E, H], F32, isOutput=False)
          for g in GROUPS}
    w2 = {g: nc.declare_dram_parameter(f"w2_{g}", [E, H, O], BF16, isOutput=False)
          for g in GROUPS}
    # packed gate params: rows 0-1023 = wg cols [t1(8), t2(8), sh(12)]
    # (sh's 12 gate cols host-permuted to [t1,sh,t2]); row 1024 = bg; padded
    # to 1152 rows so the (it p) rearrange is exact.
    wg_pk = nc.declare_dram_parameter("wg_pk", [1152, 28], BF16, isOutput=False)
    # packed fc2 biases: b2a rows = [t1(4), sh(4), t2(4)], b2b = [t2(4), sh(4)]
    b2a_d = nc.declare_dram_parameter("b2a", [12, O], F32R, isOutput=False)
    b2b_d = nc.declare_dram_parameter("b2b", [8, O], F32R, isOutput=False)
    outs = {h: nc.declare_dram_parameter(h, [BL, O], F32, isOutput=True)
            for h in ("osh", "o1", "o2")}

    with TileContext(nc) as tc:
        with tc.tile_pool(name="persist", bufs=1) as pp, \
             tc.tile_pool(name="work", bufs=1) as pw, \
             tc.tile_pool(name="ps", bufs=1, space="PSUM") as ps:
            # persistent SBUF: xT per group, gates, accumulators, consts
            xT = {g: pp.tile([128, IT, BL], BF16, name=f"xT_{g}") for g in GROUPS}
            gsb = {g: pp.tile([128, BT, GATE_W[g]], F32, name=f"g_{g}")
                   for g in GROUPS}
            acc = {h: pp.tile([128, BT, O], F32, name=f"acc_{h}")
                   for h in ("osh", "o1", "o2")}
            ident = pp.tile([128, 128], F32, name="ident")
            make_identity(nc, ident[:, :])
            ones_f = pp.tile([1, 128], F32, name="ones_f")
            nc.gpsimd.memset(ones_f[:, :], 1.0)
            ones_bf = pp.tile([1, 128], BF16, name="ones_bf")
            nc.vector.tensor_copy(ones_bf[:, :], ones_f[:, :])

            # software-pipeline prologue: first iteration's xT loads run
            # before the loop; each body reloads xT for the NEXT iteration
            # as soon as a group's fc1 blocks are done with it.
            for g in GROUPS:
                for it in range(IT):
                    nc.sync.dma_start(xT[g][:, it, :],
                                      xs[g][:, it * 128:(it + 1) * 128],
                                      transpose=True)
            loop_cm = tc.For_i(0, loop_reps, 1) if loop_reps else nullcontext()
            with loop_cm:
              for un in range(unroll):
                U = f"u{un}_" if unroll > 1 else ""
                ncp = 0

                def cp(dst, src, _un=un):  # alternate DVE / ACT drains
                    nonlocal ncp
                    if ncp % 2 == 0:
                        nc.vector.tensor_copy(dst, src)
                    else:
                        nc.scalar.copy(dst, src)
                    ncp += 1

                # ---- input loads ------------------------------------
                # xT was loaded by the previous body iteration (or the
                # prologue); only the packed small params load here, on the
                # ACT queue so the SP weight stream is undisturbed.
                wg_sb = pw.tile([128, 9, 28], BF16, tag="wgpk", bufs=1,
                                name=f"{U}wg_pk_sb")
                nc.scalar.dma_start(wg_sb[:, :, :],
                                    wg_pk.rearrange("(it p) e -> p it e", p=128))
                b2a = pw.tile([12, O], F32R, tag="b2a", bufs=1,
                              name=f"{U}b2a_sb")
                nc.scalar.dma_start(b2a[:, :], b2a_d[:, :])
                b2b = pw.tile([8, O], F32R, tag="b2b", bufs=1,
                              name=f"{U}b2b_sb")
                nc.scalar.dma_start(b2b[:, :], b2b_d[:, :])
                GCOL = {"t1": 0, "t2": 8, "sh": 16}
                BIAS_MM = {"o1": ("t1", "a", 0, 8), "o2": ("t2", "b", 0, 8),
                           "osh": ("sh", "a", 0, 12)}

                # ---- gate / head-bias task emitters ------------------
                gT = {g: pw.tile([GATE_W[g], BT, 128], F32R, tag=f"gT{g}",
                                 bufs=1, name=f"{U}gT_{g}")
                      for g in GROUPS}

                def emit_gate(g, bt, _un=un):
                    c0 = GCOL[g]
                    c1 = c0 + GATE_W[g]
                    gps = ps.tile([128, GATE_W[g]], F32,
                                  tag="gp" if gp_tag else "ph", bufs=4,
                                  name=f"{U}gps_{g}_{bt}")
                    for it in range(IT):
                        nc.tensor.matmul(
                            gps[:, :],
                            xT[g][:, it, bt * 128:(bt + 1) * 128],
                            wg_sb[:, it, c0:c1],
                            start=(it == 0), stop=False)
                    nc.tensor.matmul(gps[:, :], ones_bf[:, :],
                                     wg_sb[0:1, 8, c0:c1],
                                     start=False, stop=True)
                    # softmax over free dim
                    mx = pw.tile([128, 1], F32, tag="mx", bufs=2,
                                 name=f"{U}mx_{g}_{bt}")
                    nc.vector.reduce_max(mx[:, :], gps[:, :],
                                         axis=mybir.AxisListType.X)
                    nmx = pw.tile([128, 1], F32, tag="nmx", bufs=2,
                                  name=f"{U}nmx_{g}_{bt}")
                    nc.vector.tensor_scalar_mul(nmx[:, :], mx[:, :], -1.0)
                    ex = pw.tile([128, GATE_W[g]], F32, tag="ex", bufs=2,
                                 name=f"{U}ex_{g}_{bt}")
                    nc.scalar.activation(ex[:, :], gps[:, :],
                                         mybir.ActivationFunctionType.Exp,
                                         bias=nmx[:, :], scale=1.0)
                    sm = pw.tile([128, 1], F32, tag="sm", bufs=2,
                                 name=f"{U}sm_{g}_{bt}")
                    nc.vector.reduce_sum(sm[:, :], ex[:, :],
                                         axis=mybir.AxisListType.X)
                    rs = pw.tile([128, 1], F32, tag="rs", bufs=2,
                                 name=f"{U}rs_{g}_{bt}")
                    nc.vector.reciprocal(rs[:, :], sm[:, :])
                    nc.vector.tensor_scalar_mul(gsb[g][:, bt, :], ex[:, :],
                                                rs[:, :])
                    # transposed gates for the head-bias matmul
                    gtp = ps.tile([GATE_W[g], 128], F32,
                                  tag="gp" if gp_tag else "ph", bufs=4,
                                  name=f"{U}gtp_{g}_{bt}")
                    nc.tensor.transpose(gtp[:, :], gsb[g][:, bt, :],
                                        ident[:, :])
                    cp(gT[g][:, bt, :], gtp[:, :])

                def emit_bias(h, bt, _un=un):
                    gate, which, r0, r1 = BIAS_MM[h]
                    bsrc = b2a if which == "a" else b2b
                    pb = ps.tile([128, O], F32, tag=f"po{bt}", bufs=1,
                                 name=f"{U}pb_{h}_{bt}")
                    nc.tensor.matmul(pb[:, :], gT[gate][:, bt, :],
                                     bsrc[r0:r1, :], start=True, stop=True)
                    cp(acc[h][:, bt, :], pb[:, :])

                def emit_xload(g, it, _un=un):
                    nc.sync.dma_start(xT[g][:, it, :],
                                      xs[g][:, it * 128:(it + 1) * 128],
                                      transpose=True)

                # interleaved into the first fc1 blocks, 2 per block, in
                # dependency order; everything lands before the first
                # expert tail (~block 17) needs acc/gsb.
                tasks = (
                    [("g", "t1", bt) for bt in range(BT)]
                    + [("b", "o1", bt) for bt in range(BT)]
                    + [("g", "sh", bt) for bt in range(BT)]
                    + [("b", "osh", bt) for bt in range(BT)]
                    + [("g", "t2", bt) for bt in range(BT)]
                    + [("b", "o2", bt) for bt in range(BT)]
                )
                reloads = []

                def pop_tasks(n):
                    for _ in range(n):
                        if tasks:
                            kind, a, i = tasks.pop(0)
                            if kind == "g":
                                emit_gate(a, i)
                            else:
                                emit_bias(a, i)
                    for _ in range(2):
                        if reloads:
                            gg, it = reloads.pop(0)
                            emit_xload(gg, it)

                # ---- Phase B: experts, fc2 software-pipelined by one ----
                # PE queue is in-order; emitting mm2(ht) right after mm1(ht)
                # would stall PE on the relu(ht) dependency. Instead mm2(ht)
                # is emitted after mm1(ht+1), so the relu latency hides under
                # the next fc1 block.
                HTG = 512 // 128  # ht-tiles per W1/W2 column block
                expert_psum = {}

                def emit_mm2(g, e, ht, hT, w2t, ht4, _un=un):
                    if ht == 0:
                        expert_psum[(g, e)] = [
                            ps.tile([128, O], F32, tag=f"po{bt}", bufs=1,
                                    name=f"{U}po_{g}_{e}_{bt}")
                            for bt in range(BT)]
                    psum_o = expert_psum[(g, e)]
                    for bt in range(BT):
                        nc.tensor.matmul(
                            psum_o[bt][:, :],
                            hT[:, bt * 128:(bt + 1) * 128],
                            w2t[:, ht4, :],
                            start=(ht == 0), stop=(ht == HT - 1))
                    if ht != HT - 1:
                        return
                    # expert tail: gated accumulation straight from PSUM
                    for bt in range(BT):
                        if direct_acc:
                            src_o = psum_o[bt]
                        else:
                            src_o = pw.tile([128, O], F32, tag="o_sb", bufs=4,
                                            name=f"{U}osb_{g}{e}_{bt}")
                            nc.scalar.copy(src_o[:, :], psum_o[bt][:, :])
                        for head, gate, col in _contribs(g, e):
                            gcol = gsb[gate][:, bt, col:col + 1]
                            nc.vector.scalar_tensor_tensor(
                                acc[head][:, bt, :], src_o[:, :],
                                gcol, acc[head][:, bt, :],
                                op0=mybir.AluOpType.mult,
                                op1=mybir.AluOpType.add)

                pending = []
                SKEW = skew
                step = 0
                for gi, g in enumerate(GROUPS):
                    # previous group's xT is no longer read: reload it for
                    # the next loop iteration while this group computes
                    if gi > 0:
                        reloads.extend((GROUPS[gi - 1], it) for it in range(IT))
                    for e in range(E):
                        b1_sb = pw.tile([128, HT], F32, tag="b1", bufs=2,
                                        name=f"{U}b1_{g}{e}")
                        nc.sync.dma_start(
                            b1_sb[:, :],
                            b1[g][e].rearrange("(ht p) -> p ht", p=128))
                        for ht in range(HT):
                            htg, ht4 = divmod(ht, HTG)
                            if ht4 == 0:
                                # W1 column block [1024, 512] -> 1KB DMA beats
                                w1t = pw.tile([128, IT, 512], BF16, tag="w1",
                                              bufs=w1bufs, name=f"{U}w1_{g}{e}_{htg}")
                                if mode != "compute":
                                    nc.sync.dma_start(
                                        w1t[:, :, :],
                                        w1[g][e, :, htg * 512:(htg + 1) * 512]
                                        .rearrange("(it p) h -> p it h", p=128))
                                else:
                                    nc.sync.dma_start(
                                        w1t[:, 0, 0:1],
                                        w1[g][e, 0:128, htg * 512:htg * 512 + 1]
                                        .rearrange("p h -> p h"))
                                # W2 row block [512, 512] -> 1KB DMA beats
                                w2t = pw.tile([128, HTG, O], BF16, tag="w2",
                                              bufs=w2bufs, name=f"{U}w2_{g}{e}_{htg}")
                                if mode != "compute":
                                    nc.sync.dma_start(
                                        w2t[:, :, :],
                                        w2[g][e, htg * 512:(htg + 1) * 512, :]
                                        .rearrange("(hh p) o -> p hh o", p=128))
                                else:
                                    nc.sync.dma_start(
                                        w2t[:, 0, 0:1],
                                        w2[g][e, htg * 512:htg * 512 + 128, 0:1])

                            if mode == "dma":
                                continue
                            ph = ps.tile([128, BL], F32, tag="ph", bufs=phbufs,
                                         name=f"{U}ph_{g}{e}_{ht}")
                            for it in range(IT):
                                nc.tensor.matmul(
                                    ph[:, :],
                                    w1t[:, it, ht4 * 128:(ht4 + 1) * 128],
                                    xT[g][:, it, :],
                                    start=(it == 0),
                                    stop=(it == IT - 1))
                            hT = pw.tile([128, BL], BF16, tag="hT", bufs=htbufs,
                                         name=f"{U}hT_{g}{e}_{ht}")
                            # relu(ph + b1) -> bf16; alternate DVE/ACT to
                            # split the epilogue load across both engines
                            if step % 2 == 0:
                                nc.vector.tensor_scalar(
                                    hT[:, :], ph[:, :],
                                    b1_sb[:, ht:ht + 1], 0.0,
                                    op0=mybir.AluOpType.add,
                                    op1=mybir.AluOpType.max)
                            else:
                                nc.scalar.activation(
                                    hT[:, :], ph[:, :],
                                    mybir.ActivationFunctionType.Relu,
                                    bias=b1_sb[:, ht:ht + 1], scale=1.0)
                            pending.append((g, e, ht, hT, w2t, ht4))
                            if len(pending) > SKEW:
                                emit_mm2(*pending.pop(0))
                            pop_tasks(2)
                            step += 1
                while pending:
                    emit_mm2(*pending.pop(0))
                pop_tasks(len(tasks))
                # last group's xT reload for the next loop iteration
                for it in range(IT):
                    emit_xload(GROUPS[-1], it)
                while reloads:
                    gg, it = reloads.pop(0)
                    emit_xload(gg, it)

                # ---- store outputs (gpsimd software-DGE queue) -------
                for h in (() if mode == "dma" else ("osh", "o1", "o2")):
                    for bt in range(BT):
                        nc.gpsimd.dma_start(outs[h][bt * 128:(bt + 1) * 128, :],
                                            acc[h][:, bt, :])

    nc.finalize()
    return nc


def make_in_maps(np_in):
    """Host-side marshalling: slice the batch per core; cast the big
    operands (x, W1, W2, wg) to bf16; pack the gate weights+biases into one
    array (sh gate columns permuted to [t1, sh, t2]) and the fc2 biases
    into two stacked arrays so each lands in SBUF with a single DMA."""
    bf = ml_dtypes.bfloat16
    wcast = {}
    for g in GROUPS:
        wcast[f"w1_{g}"] = np.ascontiguousarray(np_in[f"w1_{g}"].astype(bf))
        wcast[f"w2_{g}"] = np.ascontiguousarray(np_in[f"w2_{g}"].astype(bf))
        wcast[f"b1_{g}"] = np.ascontiguousarray(np_in[f"b1_{g}"].astype(np.float32))
    sh_perm = [0, 1, 2, 3, 8, 9, 10, 11, 4, 5, 6, 7]  # [t1,t2,sh]->[t1,sh,t2]
    wg_pk = np.zeros((1152, 28), np.float32)
    wg_pk[:I, 0:8] = np_in["wg_t1"]
    wg_pk[:I, 8:16] = np_in["wg_t2"]
    wg_pk[:I, 16:28] = np_in["wg_sh"][:, sh_perm]
    wg_pk[I, 0:8] = np_in["bg_t1"]
    wg_pk[I, 8:16] = np_in["bg_t2"]
    wg_pk[I, 16:28] = np_in["bg_sh"][sh_perm]
    wcast["wg_pk"] = np.ascontiguousarray(wg_pk.astype(bf))
    b2t1 = np_in["b2_t1"].astype(np.float32)
    b2t2 = np_in["b2_t2"].astype(np.float32)
    b2sh = np_in["b2_sh"].astype(np.float32)
    wcast["b2a"] = np.ascontiguousarray(np.concatenate([b2t1, b2sh, b2t2], 0))
    wcast["b2b"] = np.ascontiguousarray(np.concatenate([b2t2, b2sh], 0))
    in_maps = []
    for c in range(N_CORES):
        sl = slice(c * BL, (c + 1) * BL)
        m = {
            "x_sh": np.ascontiguousarray(np_in["x_shared"][sl].astype(bf)),
            "x_t1": np.ascontiguousarray(np_in["x_task1"][sl].astype(bf)),
            "x_t2": np.ascontiguousarray(np_in["x_task2"][sl].astype(bf)),
        }
        m.update(wcast)
        in_maps.append(m)
    return in_maps


_NC_CACHE = None


def _get_nc():
    global _NC_CACHE
    if _NC_CACHE is None:
        _NC_CACHE = build_nc()
    return _NC_CACHE


def kernel(**inputs) -> tuple:
    from concourse.bass_utils import run_bass_kernel_spmd

    nc = _get_nc()
    np_in = {k: np.asarray(v) for k, v in inputs.items()}
    in_maps = make_in_maps(np_in)

    # rare transient NRT_EXEC_UNIT_UNRECOVERABLE crashes have been observed
    # on this fabric; retry a couple of times before giving up
    last_err = None
    for attempt in range(3):
        try:
            r = run_bass_kernel_spmd(nc, in_maps, list(range(N_CORES)))
            break
        except Exception as ex:  # noqa: BLE001
            last_err = ex
            import time as _time
            _time.sleep(5 * (attempt + 1))
    else:
        raise last_err
    out_sh = np.concatenate([r.results[c]["osh"] for c in range(N_CORES)], axis=0)
    out1 = np.concatenate([r.results[c]["o1"] for c in range(N_CORES)], axis=0)
    out2 = np.concatenate([r.results[c]["o2"] for c in range(N_CORES)], axis=0)
    return (out_sh, out1, out2)
